# revision 45
# baseline (speedup 1.0000x reference)
"""Clifford ISTA kernel for 8 Trainium2 NeuronCores — M2(C) formulation.

Strategy (data-parallel, zero cross-core communication):
  - Shard batch B=64 across 8 cores (8 per core).
  - Cl(3,0) ~= 2x2 complex matrices (Pauli rep). Each Clifford product
    becomes 2x2 complex matmul: 32 real MACs per blade-pair instead of 64
    via the Cayley table, and no 8x blade-permuted operand copies.
  - The phi(x) representation Xacc [n, (c,s,b,r)] lives persistently in
    PSUM and is updated incrementally (linearity of phi):
        Xacc += phi(-STEP*grad)  — one matmul straight off the stk tile
                                   (constant COMP = PSIE @ PHI folds the
                                   blade reconstruction + re-projection)
        Xacc += phi(-c)          — one matmul off the clamp output c_bf
    so the critical path to the next FWD pass avoids the fp32 state.
  - Per iteration: FWD (128 mm, A-stationary 128x128, moving 16 cols),
    BWD (128 mm), TRANS (4 PE transposes), PSI (2x2 mm; u = x - STEP*grad
    lands directly in PSUM via an f32r identity matmul that folds the
    fp32 state in) + DVE/ACT staging copies; c = clamp on DVE.
  - Weight loads overlap via the dual weight buffer, so matmul cost is
    dominated by moving-operand columns: A-stationary orientation gives
    ~5.6k PE cycles/iter vs ~19.5k for the Cayley-table formulation.
  - PSUM dep tracking is whole-tile: independent pipeline stages use
    separate banks (xaccA/B, psc0/1, psgA/B) and psgt/psu share banks
    via same-tag slot rotation. Exactly one start=True per PSUM bank
    per accumulation round (start zeroes the whole bank lazily).
"""

import os
import numpy as np
import ml_dtypes

bfloat16 = ml_dtypes.bfloat16

B, M, N, NB = 64, 256, 512, 8
BL = 8
NCORES = 8
N_ITER = 50
STEP = 0.01
LAMBDAS = [0.0, 0.001, 0.001, 0.002]



def _grades():
    return np.array([bin(i).count("1") for i in range(NB)], np.int32)


def _pauli():
    s1 = np.array([[0, 1], [1, 0]], np.complex64)
    s2 = np.array([[0, -1j], [1j, 0]], np.complex64)
    s3 = np.array([[1, 0], [0, -1]], np.complex64)
    I2 = np.eye(2, dtype=np.complex64)
    P = {0: I2, 1: s1, 2: s2, 4: s3,
         3: s1 @ s2, 5: s1 @ s3, 6: s2 @ s3, 7: s1 @ s2 @ s3}
    PHI8 = np.zeros((2, 2, 2, 8), np.float32)   # [c, r, s, k]
    for k in range(8):
        PHI8[0, :, :, k] = P[k].real
        PHI8[1, :, :, k] = P[k].imag
    PSI8 = np.linalg.inv(PHI8.reshape(8, 8))    # [k, (c,r,s)]
    return PHI8, PSI8


def _phi_of(v, PHI8):
    Pc = (PHI8[0] + 1j * PHI8[1]).astype(np.complex64)   # [r, s, k]
    return np.einsum('...k,rsk->...rs', v.astype(np.complex64), Pc)


def build_shared_consts(A):
    A = np.asarray(A, np.float32)
    PHI8, PSI8 = _pauli()
    grades = _grades()
    rev = ((-1.0) ** (grades * (grades - 1) // 2)).astype(np.float32)

    Ac = _phi_of(A, PHI8)                                # [M, N, s, t]
    ABc = _phi_of(STEP * A * rev[None, None, :], PHI8)   # [M, N, t, s]

    AFt = np.zeros((128, 16384), np.float32)
    Ar, Ai = Ac.real, Ac.imag
    for c in range(2):
        for q in range(4):
            for s in range(2):
                cc = c * 8 + q * 2 + s
                for h in range(2):
                    for cp in range(2):
                        for t in range(2):
                            oc = h * 4 + cp * 2 + t
                            if cp == 0:
                                blk, sg = (Ar, 1.0) if c == 0 else (Ai, -1.0)
                            else:
                                blk, sg = (Ai, 1.0) if c == 0 else (Ar, 1.0)
                            sub = blk[128 * h:128 * (h + 1),
                                      128 * q:128 * (q + 1), s, t].T
                            base = (cc * 8 + oc) * 128
                            AFt[:, base:base + 128] = sg * sub

    ABt = np.zeros((128, 16384), np.float32)
    Br, Bi = ABc.real, ABc.imag
    for c2 in range(2):
        for h in range(2):
            for t in range(2):
                cc2 = c2 * 4 + h * 2 + t
                for q in range(4):
                    for cp in range(2):
                        for s in range(2):
                            oc2 = q * 4 + cp * 2 + s
                            if cp == 0:
                                blk, sg = (Br, 1.0) if c2 == 0 else (Bi, -1.0)
                            else:
                                blk, sg = (Bi, 1.0) if c2 == 0 else (Br, 1.0)
                            sub = blk[128 * h:128 * (h + 1),
                                      128 * q:128 * (q + 1), t, s]
                            base = (cc2 * 16 + oc2) * 128
                            ABt[:, base:base + 128] = sg * sub

    PHI = np.zeros((64, 64), np.float32)
    for k in range(8):
        for b in range(8):
            for c in range(2):
                for s in range(2):
                    for r in range(2):
                        PHI[k * 8 + b, c * 32 + s * 16 + b * 2 + r] = \
                            PHI8[c, r, s, k]

    PSIE = np.zeros((128, 64), np.float32)
    for cp in range(2):
        for s in range(2):
            for b in range(8):
                for r in range(2):
                    row = cp * 32 + s * 16 + b * 2 + r
                    for k in range(8):
                        PSIE[row, k * 8 + b] = PSI8[k, cp * 4 + r * 2 + s]
    PSIE[64:128, :] = -np.eye(64, dtype=np.float32)

    COMP = PSIE @ PHI                                    # [128, 64], 0/±1

    thr_blades = np.array(LAMBDAS, np.float32)[grades]
    pthr = np.repeat(thr_blades, BL)[:, None].astype(np.float32)
    return AFt, ABt, PHI, PSIE, COMP, pthr


def build_gy(y, A):
    PHI8, PSI8 = _pauli()
    grades = _grades()
    rev = ((-1.0) ** (grades * (grades - 1) // 2)).astype(np.float32)
    Yc = _phi_of(y, PHI8)
    ARc = _phi_of(STEP * A * rev[None, None, :], PHI8)
    Yf = Yc.transpose(0, 2, 1, 3).reshape(B * 2, M * 2)
    Af = ARc.transpose(0, 2, 1, 3).reshape(M * 2, N * 2)
    Gf = (Yf @ Af).reshape(B, 2, N, 2).transpose(0, 2, 1, 3)
    parts = np.stack([Gf.real, Gf.imag], axis=2)
    G = np.einsum('bnj,kj->bnk', parts.reshape(B, N, 8), PSI8)
    out = []
    for c in range(NCORES):
        g = G[BL * c:BL * (c + 1)]
        out.append(np.ascontiguousarray(
            g.transpose(2, 0, 1).reshape(NB * BL, N)).astype(np.float32))
    return out


def build_program(n_iter=N_ITER):
    from contextlib import ExitStack
    import concourse.bass as bass
    import concourse.tile as tile
    from concourse import bacc, mybir

    f32 = mybir.dt.float32
    f32r = mybir.dt.float32r
    bf16 = mybir.dt.bfloat16
    ALU = mybir.AluOpType

    nc = bacc.Bacc(None, target_bir_lowering=False)

    aft_d = nc.dram_tensor("aft", [128, 16384], bf16, kind="ExternalInput")
    abt_d = nc.dram_tensor("abt", [128, 16384], bf16, kind="ExternalInput")
    phi_d = nc.dram_tensor("phi", [64, 64], bf16, kind="ExternalInput")
    phin_d = nc.dram_tensor("phin", [64, 64], bf16, kind="ExternalInput")
    psie_d = nc.dram_tensor("psie", [128, 64], bf16, kind="ExternalInput")
    compn_d = nc.dram_tensor("compn", [128, 64], bf16, kind="ExternalInput")
    ident_d = nc.dram_tensor("ident", [128, 128], bf16, kind="ExternalInput")
    gy_d = nc.dram_tensor("gy", [64, 512], f32, kind="ExternalInput")
    gyb_d = nc.dram_tensor("gyb", [64, 512], bf16, kind="ExternalInput")
    idf_d = nc.dram_tensor("idf", [64, 64], f32r, kind="ExternalInput")
    pthr_d = nc.dram_tensor("pthr", [64, 1], f32, kind="ExternalInput")
    nthr_d = nc.dram_tensor("nthr", [64, 1], f32, kind="ExternalInput")
    xout_d = nc.dram_tensor("xout", [64, 512], f32, kind="ExternalOutput")

    with ExitStack() as ctx:
        tc = ctx.enter_context(tile.TileContext(nc))
        cpool = ctx.enter_context(tc.tile_pool(name="consts", bufs=1))
        wpool = ctx.enter_context(tc.tile_pool(name="work", bufs=2))
        ppool = ctx.enter_context(tc.tile_pool(name="ps", bufs=1, space="PSUM"))

        pthr_t = cpool.tile([64, 1], f32, name="pthr_t")
        nc.sync.dma_start(pthr_t[:], pthr_d[:])
        nthr_t = cpool.tile([64, 1], f32, name="nthr_t")
        nc.sync.dma_start(nthr_t[:], nthr_d[:])
        gy_t = cpool.tile([64, 512], f32, name="gy_t")
        nc.sync.dma_start(gy_t[:], gy_d[:])
        phi_t = cpool.tile([64, 64], bf16, name="phi_t")
        nc.sync.dma_start(phi_t[:], phi_d[:])
        phin_t = cpool.tile([64, 64], bf16, name="phin_t")
        nc.sync.dma_start(phin_t[:], phin_d[:])
        psie_t = cpool.tile([128, 64], bf16, name="psie_t")
        nc.sync.dma_start(psie_t[:], psie_d[:])
        compn_t = cpool.tile([128, 64], bf16, name="compn_t")
        nc.sync.dma_start(compn_t[:], compn_d[:])
        ident_t = cpool.tile([128, 128], bf16, name="ident_t")
        nc.sync.dma_start(ident_t[:], ident_d[:])
        stkA = cpool.tile([128, 256], bf16, name="stkA")
        nc.sync.dma_start(stkA[64:128, :], gyb_d[:, 0:256])
        stkB = cpool.tile([128, 256], bf16, name="stkB")
        nc.sync.dma_start(stkB[64:128, :], gyb_d[:, 256:512])
        idf_t = cpool.tile([64, 64], f32r, name="idf_t")
        nc.sync.dma_start(idf_t[:], idf_d[:])
        aft_t = cpool.tile([128, 16384], bf16, name="aft_t")
        abt_t = cpool.tile([128, 16384], bf16, name="abt_t")
        for ch in range(8):
            sl = slice(2048 * ch, 2048 * (ch + 1))
            nc.sync.dma_start(aft_t[:, sl], aft_d[:, sl])
            nc.scalar.dma_start(abt_t[:, sl], abt_d[:, sl])

        x_kb = cpool.tile([64, 512], f32r, name="x_kb")   # fp32 bits (f32r)
        x1b = cpool.tile([64, 512], bf16, name="x1b")      # iter-0 bf16 x

        # 8 PSUM banks: xaccA/B (phi(x) halves, persistent accumulation),
        # psc0/1, psgA/B, psgt (partition-split 0:64 / 64:128),
        # psu (partition-split 0:64 / 64:128).
        # NOTE: dep tracking is partition-range granular — independent
        # producers/consumers need separate tiles or disjoint partitions.
        xacc = [ppool.tile([128, 128], f32, name=f"xacc{i}", tag=f"xacc{i}",
                           bufs=1) for i in range(2)]

        def xacc_sl(q):
            return xacc[q // 2][:, 64 * (q % 2):64 * (q % 2) + 64]

        def xc_copy(half, it):
            """bf16 copy of one xacc half for the next FWD; A DVE, B ACT."""
            t_ = wpool.tile([128, 128], bf16, name=f"xc{half}_{it}",
                            tag=f"xc{half}", bufs=1)
            if half == 0:
                nc.vector.tensor_copy(t_[:], xacc[0][:])
            else:
                nc.scalar.copy(t_[:], xacc[1][:])
            return t_

        # ---- iteration 0: u = GY ----
        for ch in range(2):
            sl = slice(256 * ch, 256 * (ch + 1))
            c_t = wpool.tile([64, 256], bf16, name=f"c{ch}_0", tag="c",
                             bufs=2)
            nc.vector.tensor_scalar(c_t[:], gy_t[:, sl], nthr_t[:], pthr_t[:],
                                    ALU.max, ALU.min)
            nc.vector.tensor_sub(x1b[:, sl], gy_t[:, sl], c_t[:])
            nc.gpsimd.tensor_sub(x_kb[:, sl], gy_t[:, sl], c_t[:])
            for qq in range(2):
                q = 2 * ch + qq
                # one start=True per PSUM bank: it zeroes the whole bank
                # lazily (pending-zero), later writes to fresh bytes land
                # as writes, not accumulates.
                nc.tensor.matmul(xacc_sl(q),
                                 lhsT=x1b[:, 128 * q:128 * (q + 1)],
                                 rhs=phi_t[:], start=(qq == 0), stop=False,
                                 skip_group_check=True)
        xch = [xc_copy(0, 0), xc_copy(1, 0)]

        CC2 = [(c2, h, t) for h in range(2) for c2 in range(2)
               for t in range(2)]

        # ---- iterations 1..n_iter-1 ----
        for it in range(1, n_iter):
            last = it == n_iter - 1

            def fwd_rhs(c, q, s):
                return xch[q // 2][:, (q % 2) * 64 + (c * 2 + s) * 16:
                                   (q % 2) * 64 + (c * 2 + s) * 16 + 16]

            # FWD: h-block at a time (early psc0 closure); within a block
            # ccs in q order, q3 ccs close regions last.
            psc = [ppool.tile([128, 64], f32, name=f"psc{h}_{it}",
                              tag=f"psc{h}", bufs=1) for h in range(2)]
            ec = [wpool.tile([128, 64], bf16, name=f"ec{h}_{it}",
                             tag=f"ec{h}", bufs=1) for h in range(2)]
            CC_ = [(c, q, s) for q in range(4) for c in range(2)
                   for s in range(2)]
            for h in range(2):
                for i in range(12):
                    c, q, s = CC_[i]
                    cc = c * 8 + q * 2 + s
                    for cp in range(2):
                        for t in range(2):
                            oc = h * 4 + cp * 2 + t
                            base = (cc * 8 + oc) * 128
                            # exactly one start per bank (first mm)
                            nc.tensor.matmul(
                                psc[h][:, (cp * 2 + t) * 16:
                                       (cp * 2 + t) * 16 + 16],
                                lhsT=aft_t[:, base:base + 128],
                                rhs=fwd_rhs(c, q, s),
                                start=(i == 0 and cp == 0 and t == 0),
                                stop=False)
                # q3 closure
                for cp in range(2):
                    for t in range(2):
                        for i in range(12, 16):
                            c, q, s = CC_[i]
                            cc = c * 8 + q * 2 + s
                            oc = h * 4 + cp * 2 + t
                            base = (cc * 8 + oc) * 128
                            nc.tensor.matmul(
                                psc[h][:, (cp * 2 + t) * 16:
                                       (cp * 2 + t) * 16 + 16],
                                lhsT=aft_t[:, base:base + 128],
                                rhs=fwd_rhs(c, q, s),
                                start=False,
                                stop=(i == 15 and cp == 1 and t == 1))
                # one EC copy per psc tile (readers of a tile serialize)
                nc.vector.tensor_copy(ec[h][:], psc[h][:])

            # BWD: blocks [q01-h0ccs, q23-h0ccs, q01-h1ccs(close psgA),
            # q23-h1ccs(close psgB)]
            psg = [ppool.tile([128, 128], f32, name=f"psg{i}_{it}",
                              tag=f"psg{i}", bufs=1) for i in range(2)]

            def psg_sl(q, cp, s):
                return psg[q // 2][:, (q % 2) * 64 + (cp * 2 + s) * 16:
                                   (q % 2) * 64 + (cp * 2 + s) * 16 + 16]

            def bwd_block(qpair, half, start, stop):
                qs = (0, 1) if qpair == 0 else (2, 3)
                for i in (range(4) if half == 0 else range(4, 8)):
                    c2, h, t = CC2[i]
                    cc2 = c2 * 4 + h * 2 + t
                    rhs = ec[h][:, (c2 * 2 + t) * 16:(c2 * 2 + t) * 16 + 16]
                    for q in qs:
                        for cp in range(2):
                            for s in range(2):
                                oc2 = q * 4 + cp * 2 + s
                                base = (cc2 * 16 + oc2) * 128
                                nc.tensor.matmul(
                                    psg_sl(q, cp, s),
                                    lhsT=abt_t[:, base:base + 128], rhs=rhs,
                                    start=(start and i in (0, 4)
                                           and q == qs[0] and cp == 0
                                           and s == 0),
                                    stop=(stop and i in (3, 7)
                                          and q == qs[1] and cp == 1
                                          and s == 1))

            bwd_block(0, 0, True, False)
            bwd_block(1, 0, True, False)
            bwd_block(0, 1, False, True)   # psgA closes
            # GT-A copy + TRANS q0/q1 overlap the q23 closure
            gtA = wpool.tile([128, 128], bf16, name=f"gtA_{it}",
                             tag="gtA", bufs=1)
            nc.vector.tensor_copy(gtA[:], psg[0][:])
            bwd_block(1, 1, False, True)   # psgB closes
            gtB = wpool.tile([128, 128], bf16, name=f"gtB_{it}",
                             tag="gtB", bufs=1)
            nc.vector.tensor_copy(gtB[:], psg[1][:])

            # TRANS into per-half scratch tiles (bank shared with psu via
            # same-tag slot rotation; lifetimes serialize naturally)
            psgt = [ppool.tile([64, 256], bf16, name=f"psgt{i}_{it}",
                               tag=f"scr{i}", bufs=1) for i in range(2)]
            for q in range(4):
                src = gtA if q < 2 else gtB
                nc.tensor.transpose(
                    psgt[q // 2][:, 128 * (q % 2):128 * (q % 2) + 128],
                    src[:, (q % 2) * 64:(q % 2) * 64 + 64],
                    ident_t[:])

            # stk copies: A (DVE), B (ACT)
            nc.vector.tensor_copy(stkA[0:64, :], psgt[0][:])
            nc.vector.tensor_copy(stkB[0:64, :], psgt[1][:])

            # PSI (u = x - psi(stk) in PSUM via f32r x-fold) + PREP-delta
            psu = [ppool.tile([64, 256], f32, name=f"psu{i}_{it}",
                              tag=f"scr{i}", bufs=1) for i in range(2)]
            for ch in range(2):
                stk_t = stkA if ch == 0 else stkB
                sl = slice(256 * ch, 256 * (ch + 1))
                nc.tensor.matmul(psu[ch][:], lhsT=psie_t[:],
                                 rhs=stk_t[:], start=True, stop=False,
                                 skip_group_check=True)
                nc.tensor.matmul(psu[ch][:], lhsT=idf_t[:],
                                 rhs=x_kb[:, sl], start=False, stop=True,
                                 skip_group_check=True)
                if not last:
                    for qq in range(2):
                        q = 2 * ch + qq
                        nc.tensor.matmul(
                            xacc_sl(q),
                            lhsT=stk_t[:, 128 * qq:128 * (qq + 1)],
                            rhs=compn_t[:], start=False, stop=False,
                            skip_group_check=True)

            # UPDATE (2 chunks of 256): c = clamp(u); x = u - c; + PREP-c
            # then next-iteration xc copies as soon as xacc halves close.
            c_ts = []
            for ch in range(2):
                c_t = wpool.tile([64, 256], bf16, name=f"c{ch}_{it}",
                                 tag="c", bufs=2)
                nc.vector.tensor_scalar(c_t[:], psu[ch][:], nthr_t[:],
                                        pthr_t[:], ALU.max, ALU.min)
                c_ts.append(c_t)
                if not last:
                    for qq in range(2):
                        q = 2 * ch + qq
                        nc.tensor.matmul(
                            xacc_sl(q),
                            lhsT=c_t[:, 128 * qq:128 * (qq + 1)],
                            rhs=phin_t[:], start=False, stop=False,
                            skip_group_check=True)
                    xch[ch] = xc_copy(ch, it)
            for ch in range(2):
                sl = slice(256 * ch, 256 * (ch + 1))
                nc.vector.tensor_sub(x_kb[:, sl], psu[ch][:], c_ts[ch])

        nc.sync.dma_start(xout_d[:], x_kb[:].bitcast(f32))

    nc.compile()
    return nc


_program_cache = {}


def _get_program(n_iter):
    if n_iter not in _program_cache:
        _program_cache[n_iter] = build_program(n_iter)
    return _program_cache[n_iter]


LAST_INFO = {}


def kernel(y, A, _trace=False, _n_iter=None):
    y = np.asarray(y, np.float32)
    A = np.asarray(A, np.float32)
    n_iter = N_ITER if _n_iter is None else _n_iter

    from concourse.bass_utils import run_bass_kernel_spmd

    nc = _get_program(n_iter)
    AFt, ABt, PHI, PSIE, COMP, pthr = build_shared_consts(A)
    gys = build_gy(y, A)

    in_maps = []
    for c in range(NCORES):
        in_maps.append({
            "aft": AFt.astype(bfloat16), "abt": ABt.astype(bfloat16),
            "phi": PHI.astype(bfloat16), "phin": (-PHI).astype(bfloat16),
            "psie": (-PSIE).astype(bfloat16),   # PSI computes x - psi(stk)
            "compn": (-COMP).astype(bfloat16),
            "ident": np.eye(128, dtype=np.float32).astype(bfloat16),
            "idf": np.eye(64, dtype=np.float32),
            "gy": gys[c], "gyb": gys[c].astype(bfloat16),
            "pthr": pthr, "nthr": -pthr,
        })

    try:
        res = run_bass_kernel_spmd(
            nc, in_maps, core_ids=list(range(NCORES)), trace=_trace,
        )
    except ModuleNotFoundError:
        res = run_bass_kernel_spmd(
            nc, in_maps, core_ids=list(range(NCORES)), trace=False,
        )
    LAST_INFO["exec_time_ns"] = res.exec_time_ns
    LAST_INFO["results"] = res

    x = np.zeros((B, N, NB), np.float32)
    for c in range(NCORES):
        xo = np.asarray(res.results[c]["xout"]).astype(np.float32)
        x[BL * c:BL * (c + 1)] = xo.reshape(NB, BL, N).transpose(1, 2, 0)
    return x


# revision 50
# speedup vs baseline: 1.0182x; 1.0182x over previous
"""Clifford ISTA kernel for 8 Trainium2 NeuronCores — M2(C) formulation.

Strategy (data-parallel, zero cross-core communication):
  - Shard batch B=64 across 8 cores (8 per core).
  - Cl(3,0) ~= 2x2 complex matrices (Pauli rep). Each Clifford product
    becomes 2x2 complex matmul: 32 real MACs per blade-pair instead of 64
    via the Cayley table, and no 8x blade-permuted operand copies.
  - The phi(x) representation Xacc [n, (c,s,b,r)] lives persistently in
    PSUM and is updated incrementally (linearity of phi):
        Xacc += phi(-STEP*grad)  — one matmul straight off the stk tile
                                   (constant COMP = PSIE @ PHI folds the
                                   blade reconstruction + re-projection)
        Xacc += phi(-c)          — one matmul off the clamp output c_bf
    so the critical path to the next FWD pass avoids the fp32 state.
  - Per iteration: FWD (128 mm, A-stationary 128x128, moving 16 cols),
    BWD (128 mm), TRANS (4 PE transposes), PSI (2x2 mm; u = x - STEP*grad
    lands directly in PSUM via an f32r identity matmul that folds the
    fp32 state in) + DVE/ACT staging copies; c = clamp on DVE.
  - Weight loads overlap via the dual weight buffer, so matmul cost is
    dominated by moving-operand columns: A-stationary orientation gives
    ~5.6k PE cycles/iter vs ~19.5k for the Cayley-table formulation.
  - PSUM dep tracking is whole-tile: independent pipeline stages use
    separate banks (xaccA/B, psc0/1, psgA/B) and psgt/psu share banks
    via same-tag slot rotation. Exactly one start=True per PSUM bank
    per accumulation round (start zeroes the whole bank lazily).
"""

import os
import numpy as np
import ml_dtypes

bfloat16 = ml_dtypes.bfloat16

B, M, N, NB = 64, 256, 512, 8
BL = 8
NCORES = 8
N_ITER = 50
STEP = 0.01
LAMBDAS = [0.0, 0.001, 0.001, 0.002]



def _grades():
    return np.array([bin(i).count("1") for i in range(NB)], np.int32)


def _pauli():
    s1 = np.array([[0, 1], [1, 0]], np.complex64)
    s2 = np.array([[0, -1j], [1j, 0]], np.complex64)
    s3 = np.array([[1, 0], [0, -1]], np.complex64)
    I2 = np.eye(2, dtype=np.complex64)
    P = {0: I2, 1: s1, 2: s2, 4: s3,
         3: s1 @ s2, 5: s1 @ s3, 6: s2 @ s3, 7: s1 @ s2 @ s3}
    PHI8 = np.zeros((2, 2, 2, 8), np.float32)   # [c, r, s, k]
    for k in range(8):
        PHI8[0, :, :, k] = P[k].real
        PHI8[1, :, :, k] = P[k].imag
    PSI8 = np.linalg.inv(PHI8.reshape(8, 8))    # [k, (c,r,s)]
    return PHI8, PSI8


def _phi_of(v, PHI8):
    Pc = (PHI8[0] + 1j * PHI8[1]).astype(np.complex64)   # [r, s, k]
    return np.einsum('...k,rsk->...rs', v.astype(np.complex64), Pc)


def build_shared_consts(A):
    A = np.asarray(A, np.float32)
    PHI8, PSI8 = _pauli()
    grades = _grades()
    rev = ((-1.0) ** (grades * (grades - 1) // 2)).astype(np.float32)

    Ac = _phi_of(A, PHI8)                                # [M, N, s, t]
    ABc = _phi_of(STEP * A * rev[None, None, :], PHI8)   # [M, N, t, s]

    AFt = np.zeros((128, 16384), np.float32)
    Ar, Ai = Ac.real, Ac.imag
    for c in range(2):
        for q in range(4):
            for s in range(2):
                cc = c * 8 + q * 2 + s
                for h in range(2):
                    for cp in range(2):
                        for t in range(2):
                            oc = h * 4 + cp * 2 + t
                            if cp == 0:
                                blk, sg = (Ar, 1.0) if c == 0 else (Ai, -1.0)
                            else:
                                blk, sg = (Ai, 1.0) if c == 0 else (Ar, 1.0)
                            sub = blk[128 * h:128 * (h + 1),
                                      128 * q:128 * (q + 1), s, t].T
                            base = (cc * 8 + oc) * 128
                            AFt[:, base:base + 128] = sg * sub

    ABt = np.zeros((128, 16384), np.float32)
    Br, Bi = ABc.real, ABc.imag
    for c2 in range(2):
        for h in range(2):
            for t in range(2):
                cc2 = c2 * 4 + h * 2 + t
                for q in range(4):
                    for cp in range(2):
                        for s in range(2):
                            oc2 = q * 4 + cp * 2 + s
                            if cp == 0:
                                blk, sg = (Br, 1.0) if c2 == 0 else (Bi, -1.0)
                            else:
                                blk, sg = (Bi, 1.0) if c2 == 0 else (Br, 1.0)
                            sub = blk[128 * h:128 * (h + 1),
                                      128 * q:128 * (q + 1), t, s]
                            base = (cc2 * 16 + oc2) * 128
                            ABt[:, base:base + 128] = sg * sub

    PHI = np.zeros((64, 64), np.float32)
    for k in range(8):
        for b in range(8):
            for c in range(2):
                for s in range(2):
                    for r in range(2):
                        PHI[k * 8 + b, c * 32 + s * 16 + b * 2 + r] = \
                            PHI8[c, r, s, k]

    PSIE = np.zeros((128, 64), np.float32)
    for cp in range(2):
        for s in range(2):
            for b in range(8):
                for r in range(2):
                    row = cp * 32 + s * 16 + b * 2 + r
                    for k in range(8):
                        PSIE[row, k * 8 + b] = PSI8[k, cp * 4 + r * 2 + s]
    PSIE[64:128, :] = -np.eye(64, dtype=np.float32)

    COMP = PSIE @ PHI                                    # [128, 64], 0/±1

    thr_blades = np.array(LAMBDAS, np.float32)[grades]
    pthr = np.repeat(thr_blades, BL)[:, None].astype(np.float32)
    return AFt, ABt, PHI, PSIE, COMP, pthr


def build_gy(y, A):
    PHI8, PSI8 = _pauli()
    grades = _grades()
    rev = ((-1.0) ** (grades * (grades - 1) // 2)).astype(np.float32)
    Yc = _phi_of(y, PHI8)
    ARc = _phi_of(STEP * A * rev[None, None, :], PHI8)
    Yf = Yc.transpose(0, 2, 1, 3).reshape(B * 2, M * 2)
    Af = ARc.transpose(0, 2, 1, 3).reshape(M * 2, N * 2)
    Gf = (Yf @ Af).reshape(B, 2, N, 2).transpose(0, 2, 1, 3)
    parts = np.stack([Gf.real, Gf.imag], axis=2)
    G = np.einsum('bnj,kj->bnk', parts.reshape(B, N, 8), PSI8)
    out = []
    for c in range(NCORES):
        g = G[BL * c:BL * (c + 1)]
        out.append(np.ascontiguousarray(
            g.transpose(2, 0, 1).reshape(NB * BL, N)).astype(np.float32))
    return out


def build_program(n_iter=N_ITER):
    from contextlib import ExitStack
    import concourse.bass as bass
    import concourse.tile as tile
    from concourse import bacc, mybir

    f32 = mybir.dt.float32
    f32r = mybir.dt.float32r
    bf16 = mybir.dt.bfloat16
    ALU = mybir.AluOpType

    nc = bacc.Bacc(None, target_bir_lowering=False)

    aft_d = nc.dram_tensor("aft", [128, 16384], bf16, kind="ExternalInput")
    abt_d = nc.dram_tensor("abt", [128, 16384], bf16, kind="ExternalInput")
    phi_d = nc.dram_tensor("phi", [64, 64], bf16, kind="ExternalInput")
    phin_d = nc.dram_tensor("phin", [64, 64], bf16, kind="ExternalInput")
    psie_d = nc.dram_tensor("psie", [128, 64], bf16, kind="ExternalInput")
    compn_d = nc.dram_tensor("compn", [128, 64], bf16, kind="ExternalInput")
    ident_d = nc.dram_tensor("ident", [128, 128], bf16, kind="ExternalInput")
    gy_d = nc.dram_tensor("gy", [64, 512], f32, kind="ExternalInput")
    gyb_d = nc.dram_tensor("gyb", [64, 512], bf16, kind="ExternalInput")
    idf_d = nc.dram_tensor("idf", [64, 64], f32r, kind="ExternalInput")
    pthr_d = nc.dram_tensor("pthr", [64, 1], f32, kind="ExternalInput")
    nthr_d = nc.dram_tensor("nthr", [64, 1], f32, kind="ExternalInput")
    xout_d = nc.dram_tensor("xout", [64, 512], f32, kind="ExternalOutput")

    with ExitStack() as ctx:
        tc = ctx.enter_context(tile.TileContext(nc))
        cpool = ctx.enter_context(tc.tile_pool(name="consts", bufs=1))
        wpool = ctx.enter_context(tc.tile_pool(name="work", bufs=2))
        ppool = ctx.enter_context(tc.tile_pool(name="ps", bufs=1, space="PSUM"))

        pthr_t = cpool.tile([64, 1], f32, name="pthr_t")
        nc.sync.dma_start(pthr_t[:], pthr_d[:])
        nthr_t = cpool.tile([64, 1], f32, name="nthr_t")
        nc.sync.dma_start(nthr_t[:], nthr_d[:])
        gy_t = cpool.tile([64, 512], f32, name="gy_t")
        nc.sync.dma_start(gy_t[:], gy_d[:])
        phi_t = cpool.tile([64, 64], bf16, name="phi_t")
        nc.sync.dma_start(phi_t[:], phi_d[:])
        phin_t = cpool.tile([64, 64], bf16, name="phin_t")
        nc.sync.dma_start(phin_t[:], phin_d[:])
        psie_t = cpool.tile([128, 64], bf16, name="psie_t")
        nc.sync.dma_start(psie_t[:], psie_d[:])
        compn_t = cpool.tile([128, 64], bf16, name="compn_t")
        nc.sync.dma_start(compn_t[:], compn_d[:])
        ident_t = cpool.tile([128, 128], bf16, name="ident_t")
        nc.sync.dma_start(ident_t[:], ident_d[:])
        stkA = cpool.tile([128, 256], bf16, name="stkA")
        nc.sync.dma_start(stkA[64:128, :], gyb_d[:, 0:256])
        stkB = cpool.tile([128, 256], bf16, name="stkB")
        nc.sync.dma_start(stkB[64:128, :], gyb_d[:, 256:512])
        idf_t = cpool.tile([64, 64], f32r, name="idf_t")
        nc.sync.dma_start(idf_t[:], idf_d[:])
        aft_t = cpool.tile([128, 16384], bf16, name="aft_t")
        abt_t = cpool.tile([128, 16384], bf16, name="abt_t")
        for ch in range(8):
            sl = slice(2048 * ch, 2048 * (ch + 1))
            if ch % 2 == 0:
                nc.sync.dma_start(aft_t[:, sl], aft_d[:, sl])
            else:
                nc.gpsimd.dma_start(aft_t[:, sl], aft_d[:, sl])
            nc.scalar.dma_start(abt_t[:, sl], abt_d[:, sl])

        x_kb = cpool.tile([64, 512], f32r, name="x_kb")   # fp32 bits (f32r)
        x1b = cpool.tile([64, 512], bf16, name="x1b")      # iter-0 bf16 x

        # 8 PSUM banks: xaccA/B (phi(x) halves, persistent accumulation),
        # psc0/1, psgA/B, psgt (partition-split 0:64 / 64:128),
        # psu (partition-split 0:64 / 64:128).
        # NOTE: dep tracking is partition-range granular — independent
        # producers/consumers need separate tiles or disjoint partitions.
        xacc = [ppool.tile([128, 128], f32, name=f"xacc{i}", tag=f"xacc{i}",
                           bufs=1) for i in range(2)]

        def xacc_sl(q):
            return xacc[q // 2][:, 64 * (q % 2):64 * (q % 2) + 64]

        def xc_copy(half, it):
            """bf16 copy of one xacc half for the next FWD; A DVE, B ACT."""
            t_ = wpool.tile([128, 128], bf16, name=f"xc{half}_{it}",
                            tag=f"xc{half}", bufs=1)
            if half == 0:
                nc.vector.tensor_copy(t_[:], xacc[0][:])
            else:
                nc.scalar.copy(t_[:], xacc[1][:])
            return t_

        # ---- iteration 0: u = GY ----
        for ch in range(2):
            sl = slice(256 * ch, 256 * (ch + 1))
            c_t = wpool.tile([64, 256], bf16, name=f"c{ch}_0", tag="c",
                             bufs=2)
            nc.vector.tensor_scalar(c_t[:], gy_t[:, sl], nthr_t[:], pthr_t[:],
                                    ALU.max, ALU.min)
            nc.vector.tensor_sub(x1b[:, sl], gy_t[:, sl], c_t[:])
            nc.gpsimd.tensor_sub(x_kb[:, sl], gy_t[:, sl], c_t[:])
            for qq in range(2):
                q = 2 * ch + qq
                # one start=True per PSUM bank: it zeroes the whole bank
                # lazily (pending-zero), later writes to fresh bytes land
                # as writes, not accumulates.
                nc.tensor.matmul(xacc_sl(q),
                                 lhsT=x1b[:, 128 * q:128 * (q + 1)],
                                 rhs=phi_t[:], start=(qq == 0), stop=False,
                                 skip_group_check=True)
        xch = [xc_copy(0, 0), xc_copy(1, 0)]

        CC2 = [(c2, h, t) for h in range(2) for c2 in range(2)
               for t in range(2)]

        # ---- iterations 1..n_iter-1 ----
        for it in range(1, n_iter):
            last = it == n_iter - 1

            def fwd_rhs(c, q, s):
                return xch[q // 2][:, (q % 2) * 64 + (c * 2 + s) * 16:
                                   (q % 2) * 64 + (c * 2 + s) * 16 + 16]

            # FWD: h-block at a time (early psc0 closure); within a block
            # ccs in q order, q3 ccs close regions last.
            psc = [ppool.tile([128, 64], f32, name=f"psc{h}_{it}",
                              tag=f"psc{h}", bufs=1) for h in range(2)]
            ec = [wpool.tile([128, 64], bf16, name=f"ec{h}_{it}",
                             tag=f"ec{h}", bufs=1) for h in range(2)]
            CC_ = [(c, q, s) for q in range(4) for c in range(2)
                   for s in range(2)]
            for h in range(2):
                for i in range(12):
                    c, q, s = CC_[i]
                    cc = c * 8 + q * 2 + s
                    for cp in range(2):
                        for t in range(2):
                            oc = h * 4 + cp * 2 + t
                            base = (cc * 8 + oc) * 128
                            # exactly one start per bank (first mm)
                            nc.tensor.matmul(
                                psc[h][:, (cp * 2 + t) * 16:
                                       (cp * 2 + t) * 16 + 16],
                                lhsT=aft_t[:, base:base + 128],
                                rhs=fwd_rhs(c, q, s),
                                start=(i == 0 and cp == 0 and t == 0),
                                stop=False)
                # q3 closure
                for cp in range(2):
                    for t in range(2):
                        for i in range(12, 16):
                            c, q, s = CC_[i]
                            cc = c * 8 + q * 2 + s
                            oc = h * 4 + cp * 2 + t
                            base = (cc * 8 + oc) * 128
                            nc.tensor.matmul(
                                psc[h][:, (cp * 2 + t) * 16:
                                       (cp * 2 + t) * 16 + 16],
                                lhsT=aft_t[:, base:base + 128],
                                rhs=fwd_rhs(c, q, s),
                                start=False,
                                stop=(i == 15 and cp == 1 and t == 1))
                # one EC copy per psc tile (readers of a tile serialize)
                nc.vector.tensor_copy(ec[h][:], psc[h][:])

            # BWD: blocks [q01-h0ccs, q23-h0ccs, q01-h1ccs(close psgA),
            # q23-h1ccs(close psgB)]
            psg = [ppool.tile([128, 128], f32, name=f"psg{i}_{it}",
                              tag=f"psg{i}", bufs=1) for i in range(2)]

            def psg_sl(q, cp, s):
                return psg[q // 2][:, (q % 2) * 64 + (cp * 2 + s) * 16:
                                   (q % 2) * 64 + (cp * 2 + s) * 16 + 16]

            def bwd_block(qpair, half, start, stop):
                qs = (0, 1) if qpair == 0 else (2, 3)
                for i in (range(4) if half == 0 else range(4, 8)):
                    c2, h, t = CC2[i]
                    cc2 = c2 * 4 + h * 2 + t
                    rhs = ec[h][:, (c2 * 2 + t) * 16:(c2 * 2 + t) * 16 + 16]
                    for q in qs:
                        for cp in range(2):
                            for s in range(2):
                                oc2 = q * 4 + cp * 2 + s
                                base = (cc2 * 16 + oc2) * 128
                                nc.tensor.matmul(
                                    psg_sl(q, cp, s),
                                    lhsT=abt_t[:, base:base + 128], rhs=rhs,
                                    start=(start and i in (0, 4)
                                           and q == qs[0] and cp == 0
                                           and s == 0),
                                    stop=(stop and i in (3, 7)
                                          and q == qs[1] and cp == 1
                                          and s == 1))

            bwd_block(0, 0, True, False)
            bwd_block(0, 1, False, True)   # psgA closes early
            bwd_block(1, 0, True, False)
            # GT-A copy + TRANS q0/q1 overlap the q23 closure
            gtA = wpool.tile([128, 128], bf16, name=f"gtA_{it}",
                             tag="gtA", bufs=1)
            nc.vector.tensor_copy(gtA[:], psg[0][:])
            bwd_block(1, 1, False, True)   # psgB closes
            gtB = wpool.tile([128, 128], bf16, name=f"gtB_{it}",
                             tag="gtB", bufs=1)
            nc.vector.tensor_copy(gtB[:], psg[1][:])

            # TRANS into per-half scratch tiles (bank shared with psu via
            # same-tag slot rotation; lifetimes serialize naturally)
            psgt = [ppool.tile([64, 256], bf16, name=f"psgt{i}_{it}",
                               tag=f"scr{i}", bufs=1) for i in range(2)]
            for q in range(4):
                src = gtA if q < 2 else gtB
                nc.tensor.transpose(
                    psgt[q // 2][:, 128 * (q % 2):128 * (q % 2) + 128],
                    src[:, (q % 2) * 64:(q % 2) * 64 + 64],
                    ident_t[:])

            # stk copies: A (DVE), B (ACT)
            nc.vector.tensor_copy(stkA[0:64, :], psgt[0][:])
            nc.vector.tensor_copy(stkB[0:64, :], psgt[1][:])

            # PSI (u = x - psi(stk) in PSUM via f32r x-fold) + PREP-delta
            psu = [ppool.tile([64, 256], f32, name=f"psu{i}_{it}",
                              tag=f"scr{i}", bufs=1) for i in range(2)]
            for ch in range(2):
                stk_t = stkA if ch == 0 else stkB
                sl = slice(256 * ch, 256 * (ch + 1))
                nc.tensor.matmul(psu[ch][:], lhsT=psie_t[:],
                                 rhs=stk_t[:], start=True, stop=False,
                                 skip_group_check=True)
                nc.tensor.matmul(psu[ch][:], lhsT=idf_t[:],
                                 rhs=x_kb[:, sl], start=False, stop=True,
                                 skip_group_check=True)
                if not last:
                    for qq in range(2):
                        q = 2 * ch + qq
                        nc.tensor.matmul(
                            xacc_sl(q),
                            lhsT=stk_t[:, 128 * qq:128 * (qq + 1)],
                            rhs=compn_t[:], start=False, stop=False,
                            skip_group_check=True)

            # UPDATE (2 chunks of 256): c = clamp(u); x = u - c; + PREP-c
            # then next-iteration xc copies as soon as xacc halves close.
            c_ts = []
            for ch in range(2):
                c_t = wpool.tile([64, 256], bf16, name=f"c{ch}_{it}",
                                 tag="c", bufs=2)
                nc.vector.tensor_scalar(c_t[:], psu[ch][:], nthr_t[:],
                                        pthr_t[:], ALU.max, ALU.min)
                c_ts.append(c_t)
                if not last:
                    for qq in range(2):
                        q = 2 * ch + qq
                        nc.tensor.matmul(
                            xacc_sl(q),
                            lhsT=c_t[:, 128 * qq:128 * (qq + 1)],
                            rhs=phin_t[:], start=False, stop=False,
                            skip_group_check=True)
                    xch[ch] = xc_copy(ch, it)
            for ch in range(2):
                sl = slice(256 * ch, 256 * (ch + 1))
                nc.vector.tensor_sub(x_kb[:, sl], psu[ch][:], c_ts[ch])

        nc.sync.dma_start(xout_d[:], x_kb[:].bitcast(f32))

    nc.compile()
    return nc


_program_cache = {}


def _get_program(n_iter):
    if n_iter not in _program_cache:
        _program_cache[n_iter] = build_program(n_iter)
    return _program_cache[n_iter]


LAST_INFO = {}


def kernel(y, A, _trace=False, _n_iter=None):
    y = np.asarray(y, np.float32)
    A = np.asarray(A, np.float32)
    n_iter = N_ITER if _n_iter is None else _n_iter

    from concourse.bass_utils import run_bass_kernel_spmd

    nc = _get_program(n_iter)
    AFt, ABt, PHI, PSIE, COMP, pthr = build_shared_consts(A)
    gys = build_gy(y, A)

    in_maps = []
    for c in range(NCORES):
        in_maps.append({
            "aft": AFt.astype(bfloat16), "abt": ABt.astype(bfloat16),
            "phi": PHI.astype(bfloat16), "phin": (-PHI).astype(bfloat16),
            "psie": (-PSIE).astype(bfloat16),   # PSI computes x - psi(stk)
            "compn": (-COMP).astype(bfloat16),
            "ident": np.eye(128, dtype=np.float32).astype(bfloat16),
            "idf": np.eye(64, dtype=np.float32),
            "gy": gys[c], "gyb": gys[c].astype(bfloat16),
            "pthr": pthr, "nthr": -pthr,
        })

    try:
        res = run_bass_kernel_spmd(
            nc, in_maps, core_ids=list(range(NCORES)), trace=_trace,
        )
    except ModuleNotFoundError:
        res = run_bass_kernel_spmd(
            nc, in_maps, core_ids=list(range(NCORES)), trace=False,
        )
    LAST_INFO["exec_time_ns"] = res.exec_time_ns
    LAST_INFO["results"] = res

    x = np.zeros((B, N, NB), np.float32)
    for c in range(NCORES):
        xo = np.asarray(res.results[c]["xout"]).astype(np.float32)
        x[BL * c:BL * (c + 1)] = xo.reshape(NB, BL, N).transpose(1, 2, 0)
    return x
